# revision 9
# baseline (speedup 1.0000x reference)
"""Trainium2 Bass kernel: distance-decay double-softmax attention.

Reference computation per (b, c) pair (L=256, D=512):
    qkv  = x @ w_qkv;  q,k,v = split(qkv)
    attn = softmax(q @ k.T * D_h^-0.5)
    h    = relu((attn + pos) @ w1 + b1);  w = h @ w2 + b2
    attn2= softmax(attn * exp(-dist / (2 w^2 + 1e-6)))
    out  = (attn2 @ v) @ w_out + b_out

Host-side algebraic folds (exact):
    dots = q k^T * s = x (s Wq Wk^T) x^T         -> M = s*Wq@Wk.T
    y    = attn2 @ (v w_out) + b_out             -> Wv' = Wv@w_out (b_out on host)
    (attn+pos) @ w1 + b1 = attn@w1 + (pos@w1+b1) -> P1[c] = pos[c]@w1+b1

Mixed precision (validated vs fp64 numpy, rel_err ~2.4e-3 << 2e-2 gate):
    fp8e4m3 DoubleRow matmuls for the softmax/MLP chain (t = x@(64M),
    dots = (8t)@x^T, h = (16attn)@(16w1), w = (4h)@(16w2)) -- halves the
    PE instruction count where ALU work dominates.  bf16 for the
    output-linear chain (v = x@Wv', y = attn2@v) and all attention
    tensors; scales unwound in activation `scale` params.

Engine split (balancing measured busy times): PE matmuls/transposes;
ACT exps + row-sum accumulators + y copy-scale; DVE the PSUM casts
(t8, v, a8, relu-h8, a2T) + reciprocals; GpSimd (otherwise idle) the
SBUF-only attn normalize and attn*wg multiplies via tensor_tensor.

Sharding: pure data parallel over the 128 (b,c) pairs -> 16 pairs/core,
packed as 8 "superpairs" (2 batch items of one channel share the free
dim, giving N=512 matmuls).  Emission is software-pipelined across
superpairs so the TensorEngine never drains during the softmax/MLP
chain and the HAM clock stays warm.
"""

import sys
import numpy as np

sys.path.insert(0, "/opt/trn_rl_repo")

import concourse.bass as bass  # noqa: E402,F401
import concourse.mybir as mybir  # noqa: E402
from concourse import bacc  # noqa: E402
from concourse.tile import TileContext  # noqa: E402

F32 = mybir.dt.float32
BF16 = mybir.dt.bfloat16
F8 = mybir.dt.float8e4
AF = mybir.ActivationFunctionType
ALU = mybir.AluOpType
DR = mybir.MatmulPerfMode.DoubleRow

B, C, L, D = 8, 16, 256, 512
NCORES = 8
CH_PER_CORE = C // NCORES          # 2
NSP = (B // 2) * CH_PER_CORE       # 8 superpairs per core
P = 128
FP = 2 * L                         # 512: two pairs packed along free dim
DT = D // P                        # 4
LT = L // P                        # 2
SCALE = float(64 ** -0.5)          # DIM_HEAD ** -0.5

# engine choices (flip to "dve" if GpSimd tensor_tensor disappoints)
ATTN_ENGINE = "gpsimd"
P2_ENGINE = "gpsimd"


class _Ctx:
    pass


def _emit_stage_a(g, sp, mid_hook=None, split=0):
    """x load, t8 = cast(x@64M)/8, v = bf16(x Wv'), dots=(8t)@x^T -> E, attn."""
    nc, pp, sp_pool = g.nc, g.pp, g.apool
    MM = nc.tensor.matmul
    if split == 2:
        st = g.state[sp]
        xt, x8, t8 = st.xt, st.x8, st.t8
    else:
        st = g.state[sp] = _Ctx()

        # x^T tiles bf16 [128(d), 512(l packed)] for the v matmul
        xt = []
        for dt in range(DT):
            t = sp_pool.tile([P, FP], BF16, tag=f"xt{dt}", name=f"xt{sp}_{dt}")
            nc.sync.dma_start(out=t[:, :],
                              in_=g.h["x_t"][sp, dt * P:(dt + 1) * P, :])
            xt.append(t)
        st.xt = xt

        # x^T fp8 pair-packed [128, 2(ktile), 512] per 256-wide d-pair
        x8 = []
        for dp in range(2):
            t = sp_pool.tile([P, 2, FP], F8, tag=f"x8{dp}", name=f"x8{sp}_{dp}")
            nc.gpsimd.dma_start(out=t[:, :, :], in_=g.h["x8"][sp, dp, :, :, :])
            x8.append(t)
        st.x8 = x8

        # t^T[e, l] = sum_d 64M[d, e] x^T[d, l], DoubleRow; cast to 8t fp8
        t8 = [sp_pool.tile([P, 2, FP], F8, tag=f"t8{ep}", name=f"t8{sp}_{ep}")
              for ep in range(2)]
        for et in range(DT):
            ps = pp.tile([P, FP], F32, tag="ps", name=f"ps_t{sp}_{et}")
            for dp in range(2):
                MM(ps[:, :], g.m8_sb[dp][:, :, et * P:(et + 1) * P],
                   x8[dp][:, :, :], start=(dp == 0), stop=(dp == 1),
                   perf_mode=DR)
            nc.vector.tensor_scalar_mul(t8[et // 2][:, et % 2, :], ps[:, :],
                                        0.125)
        st.t8 = t8
        if mid_hook is not None:
            mid_hook()

        # v[l, e] = sum_d x^T[d, l] Wv'[d, e] (bf16), cast bf16 on DVE
        v_sb = [[None] * LT for _ in range(2)]
        for pi in range(2):
            for lt in range(LT):
                ps = pp.tile([P, D], F32, tag="ps", name=f"ps_v{sp}_{pi}{lt}")
                for dt in range(DT):
                    MM(ps[:, :],
                       xt[dt][:, pi * L + lt * P: pi * L + (lt + 1) * P],
                       g.wv_sb[dt][:, :],
                       start=(dt == 0), stop=(dt == DT - 1))
                t = sp_pool.tile([P, D], BF16, tag=f"v{pi}{lt}",
                                 name=f"v{sp}_{pi}{lt}")
                nc.vector.tensor_copy(t[:, :], ps[:, :])
                v_sb[pi][lt] = t
        st.v = v_sb
    if split == 1:
        return

    # dots[i, m] = sum_e 8t^T[e, i] x^T[e, m], DoubleRow (scale 1/8 in exp)
    dps = []
    for it in range(LT):
        ps = pp.tile([P, FP], F32, tag="ps", name=f"ps_d{sp}_{it}")
        for pi in range(2):
            o = ps[:, pi * L:(pi + 1) * L]
            for ep in range(2):
                MM(o,
                   t8[ep][:, :, pi * L + it * P: pi * L + (it + 1) * P],
                   x8[ep][:, :, pi * L:(pi + 1) * L],
                   start=(ep == 0), stop=(ep == 1), perf_mode=DR)
        dps.append(ps)

    # E = exp(dots/8) bf16, s1 = rowsum(E); attn16 = E * (16/s1) in bf16
    s14 = sp_pool.tile([P, 4], F32, tag="s14", name=f"s14_{sp}")
    E = []
    for it in range(LT):
        e_t = sp_pool.tile([P, FP], BF16, tag=f"E{it}", name=f"E{sp}_{it}")
        for pi in range(2):
            c = it * 2 + pi
            sl = slice(pi * L, (pi + 1) * L)
            nc.scalar.activation(e_t[:, sl], dps[it][:, sl], AF.Exp,
                                 scale=0.125, accum_out=s14[:, c:c + 1])
        E.append(e_t)
    r16 = sp_pool.tile([P, 4], F32, tag="r16", name=f"r16_{sp}")
    nc.vector.tensor_scalar_mul(r16[:, :], s14[:, :], 0.0625)
    nc.vector.reciprocal(r16[:, :], r16[:, :])

    attn = []
    for it in range(LT):
        t = sp_pool.tile([P, FP], BF16, tag=f"at{it}", name=f"attn{sp}_{it}")
        if ATTN_ENGINE == "gpsimd":
            rbc = r16[:, 2 * it:2 * it + 2].broadcast_to([P, 2, L])
            nc.gpsimd.tensor_mul(t[:, :], E[it][:, :], rbc)
        else:
            for pi in range(2):
                c = it * 2 + pi
                sl = slice(pi * L, (pi + 1) * L)
                nc.vector.tensor_scalar_mul(t[:, sl], E[it][:, sl],
                                            r16[:, c:c + 1])
        attn.append(t)
    st.attn = attn


def _emit_stage_b1(g, sp):
    """transpose attn16->fp8, MLP (DoubleRow), dist-decay, softmax2 -> E2."""
    nc, pp, sp_pool = g.nc, g.pp, g.sp_pool
    MM = nc.tensor.matmul
    st = g.state[sp]
    ci = sp // (NSP // CH_PER_CORE)
    attn = st.attn

    # attn^T fp8 pair-packed [m(part), 2(mt), i(packed free)]
    a8 = sp_pool.tile([P, 2, FP], F8, tag="a8", name=f"a8_{sp}")
    for mt in range(LT):
        ps = g.ppb.tile([P, FP], BF16, tag="psb", name=f"ps_tA{sp}_{mt}")
        for pi in range(2):
            for it in range(LT):
                nc.tensor.transpose(
                    ps[:, pi * L + it * P: pi * L + (it + 1) * P],
                    attn[it][:, pi * L + mt * P: pi * L + (mt + 1) * P],
                    g.idb_sb[:, :])
        nc.vector.tensor_copy(a8[:, mt, :], ps[:, :])

    # h^T = relu((16w1)^T (16attn)^T + 256 P1^T)/64 * 4 -> fp8 (DVE)
    h8 = sp_pool.tile([P, 2, FP], F8, tag="h8", name=f"h8_{sp}")
    for jt in range(LT):
        ps = pp.tile([P, FP], F32, tag="ps", name=f"ps_h{sp}_{jt}")
        for pi in range(2):
            o = ps[:, pi * L:(pi + 1) * L]
            MM(o, g.w18_sb[:, :, jt * P:(jt + 1) * P],
               a8[:, :, pi * L:(pi + 1) * L],
               start=True, stop=False, perf_mode=DR)
        MM(ps[:, :], g.idb_sb[:, :], g.p1_sb[ci][jt][:, :],
           start=False, stop=True)
        nc.vector.tensor_scalar(h8[:, jt, :], ps[:, :], 0.015625, 0.0,
                                ALU.mult, ALU.max)

    # w[i]*64 = (4h)[i, :] @ (16w2); negt = -1/(2(w+b2)^2 + 1e-6)
    wps = pp.tile([P, 8], F32, tag="ps", name=f"ps_w{sp}")
    for pi in range(2):
        for it in range(LT):
            c = it * 2 + pi
            MM(wps[:, 2 * c:2 * c + 2],
               h8[:, :, pi * L + it * P: pi * L + (it + 1) * P],
               g.w28_sb[:, :, :],
               start=True, stop=True, perf_mode=DR)
    w4 = sp_pool.tile([P, 8], F32, tag="w4", name=f"w4_{sp}")
    nc.scalar.activation(w4[:, :], wps[:, :], AF.Square, scale=0.015625,
                         bias=g.b2_sb[:, 0:1])
    nc.vector.tensor_scalar(w4[:, :], w4[:, :], -2.0, -1e-6, ALU.mult, ALU.add)
    negt = sp_pool.tile([P, 8], F32, tag="negt", name=f"negt_{sp}")
    nc.vector.reciprocal(negt[:, :], w4[:, :])

    # wg = exp(dist*negt) bf16; p2 = attn16*wg (GpSimd); E2 = exp(p2/16) (+s2)
    s24 = sp_pool.tile([P, 4], F32, tag="s24", name=f"s24_{sp}")
    wg = []
    for it in range(LT):
        t = sp_pool.tile([P, FP], BF16, tag=f"wg{it}", name=f"wg{sp}_{it}")
        for pi in range(2):
            c = it * 2 + pi
            sl = slice(pi * L, (pi + 1) * L)
            nc.scalar.activation(t[:, sl], g.dist_sb[it][:, sl], AF.Exp,
                                 scale=negt[:, 2 * c:2 * c + 1])
        if P2_ENGINE == "gpsimd":
            nc.gpsimd.tensor_mul(t[:, :], attn[it][:, :], t[:, :])
        else:
            nc.vector.tensor_mul(t[:, :], attn[it][:, :], t[:, :])
        for pi in range(2):
            c = it * 2 + pi
            sl = slice(pi * L, (pi + 1) * L)
            nc.scalar.activation(t[:, sl], t[:, sl], AF.Exp, scale=0.0625,
                                 accum_out=s24[:, c:c + 1])
        wg.append(t)
    r24 = sp_pool.tile([P, 4], F32, tag="r24", name=f"r24_{sp}")
    nc.vector.reciprocal(r24[:, :], s24[:, :])
    st.r24 = r24
    st.wg = wg


def _emit_stage_b2(g, sp):
    """transpose E2 (bf16), y = r2 * (E2 @ v) via ACT copy-scale, DMA out."""
    nc, pp, sp_pool = g.nc, g.pp, g.sp_pool
    MM = nc.tensor.matmul
    st = g.state[sp]
    wg = st.wg

    # E2^T bf16 [m(part), i(packed)]
    a2T = []
    for mt in range(LT):
        ps = g.ppb.tile([P, FP], BF16, tag="psb", name=f"ps_tB{sp}_{mt}")
        for pi in range(2):
            for it in range(LT):
                nc.tensor.transpose(
                    ps[:, pi * L + it * P: pi * L + (it + 1) * P],
                    wg[it][:, pi * L + mt * P: pi * L + (mt + 1) * P],
                    g.idb_sb[:, :])
        t = sp_pool.tile([P, FP], BF16, tag=f"trT{mt}", name=f"a2T{sp}_{mt}")
        nc.vector.tensor_copy(t[:, :], ps[:, :])
        a2T.append(t)

    # y[i, d] = r2[i] * sum_m E2^T[m, i] v[m, d]
    for pi in range(2):
        for it in range(LT):
            c = it * 2 + pi
            ps = pp.tile([P, FP], F32, tag="ps", name=f"ps_y{sp}_{pi}{it}")
            for mt in range(LT):
                MM(ps[:, :],
                   a2T[mt][:, pi * L + it * P: pi * L + (it + 1) * P],
                   st.v[pi][mt][:, :],
                   start=(mt == 0), stop=(mt == LT - 1))
            yt = g.ypool.tile([P, FP], BF16, tag=f"y{pi}{it}",
                              name=f"y{sp}_{pi}{it}")
            if sp < NSP - 3:
                nc.scalar.activation(yt[:, :], ps[:, :], AF.Copy,
                                     scale=st.r24[:, c:c + 1])
            else:
                nc.vector.tensor_scalar_mul(yt[:, :], ps[:, :],
                                            st.r24[:, c:c + 1])
            eng = nc.sync if (pi + it) % 2 == 0 else nc.scalar
            eng.dma_start(
                out=g.h["out"][sp, pi * L + it * P: pi * L + (it + 1) * P, :],
                in_=yt[:, :])


def _emit(nc, tc, h):
    import contextlib
    g = _Ctx()
    g.nc, g.h = nc, h
    g.state = {}

    with contextlib.ExitStack() as ex:
        cpool = ex.enter_context(tc.tile_pool(name="consts", bufs=1))
        g.apool = ex.enter_context(tc.tile_pool(name="astream", bufs=4))
        g.sp_pool = ex.enter_context(tc.tile_pool(name="stream", bufs=2))
        g.ypool = ex.enter_context(tc.tile_pool(name="yout", bufs=1))
        g.pp = ex.enter_context(tc.tile_pool(name="ps", bufs=6, space="PSUM"))
        g.ppb = ex.enter_context(tc.tile_pool(name="psb", bufs=2, space="PSUM"))

        # ---- constants ----
        def cload(name, shape, dt_, src, eng=None):
            t = cpool.tile(shape, dt_, tag=name, name=name)
            t_ap = t[:, :, :] if len(shape) == 3 else t[:shape[0], :]
            (eng or nc.sync).dma_start(out=t_ap, in_=src)
            return t

        # Warmup from a memset tile (no DMA dependency -> PE starts at
        # t~0): the HAM clock gate needs ~3.4us of sustained PE activity to
        # lift the 1.2GHz cold throttle; warm it up while input DMAs land.
        warm_in = cpool.tile([P, P], BF16, tag="warm_in", name="warm_in")
        nc.vector.memset(warm_in[:, :], 1.0)
        warm_ps = g.pp.tile([P, P], F32, tag="ps", name="warmup_ps")
        for wi in range(40):
            nc.tensor.matmul(warm_ps[:, :], warm_in[:, :], warm_in[:, :],
                             start=True, stop=True)
        g.idb_sb = cload("identb", [P, P], BF16, h["identb"][:, :])

        # Stage-A consts first so the PE can start as soon as m8/x land;
        # everything stage-B needs streams in behind the first A stages.
        g.m8_sb = [cload(f"m8_{dp}", [P, 2, D], F8, h["m8"][dp, :, :, :])
                   for dp in range(2)]

        def late_consts():
            g.w18_sb = cload("w18", [P, 2, L], F8, h["w18"][:, :, :],
                             eng=nc.scalar)
            g.w28_sb = cload("w28", [P, 2, 2], F8, h["w28"][:, :, :],
                             eng=nc.scalar)
            g.p1_sb = [[cload(f"p1_{ci}_{jt}", [P, FP], BF16,
                              h["p1t"][ci, jt * P:(jt + 1) * P, :],
                              eng=nc.scalar)
                        for jt in range(LT)] for ci in range(CH_PER_CORE)]
            g.b2_sb = cload("b2r", [P, 1], F32, h["b2r"][:, :], eng=nc.scalar)
            g.dist_sb = [cload(f"dist{it}", [P, FP], F32,
                               h["dist"][it * P:(it + 1) * P, :],
                               eng=nc.scalar)
                         for it in range(LT)]

        # ---- software-pipelined superpair loop ----
        def load_wv():
            g.wv_sb = [cload(f"wv{dt}", [P, D], BF16,
                             h["wv"][dt * P:(dt + 1) * P, :])
                       for dt in range(DT)]

        _emit_stage_a(g, 0, mid_hook=load_wv)
        late_consts()
        _emit_stage_a(g, 1)
        _emit_stage_a(g, 2)
        for sp in range(NSP - 4):
            _emit_stage_b1(g, sp)
            _emit_stage_a(g, sp + 3)
            _emit_stage_b2(g, sp)
        # tail: split A(7) so its halves cover sp=4/5's chains; B1(7) before
        # B2(6) so its matmuls cover B2(6)'s wait, filler covers B2(7)'s.
        _emit_stage_b1(g, NSP - 4)
        _emit_stage_a(g, NSP - 1, split=1)          # x, t8, v only
        _emit_stage_b2(g, NSP - 4)
        _emit_stage_b1(g, NSP - 3)
        _emit_stage_a(g, NSP - 1, split=2)          # dots, exp1, attn
        _emit_stage_b2(g, NSP - 3)
        _emit_stage_b1(g, NSP - 2)
        _emit_stage_b1(g, NSP - 1)
        _emit_stage_b2(g, NSP - 2)
        fill_ps = g.pp.tile([P, FP], F32, tag="ps", name="fill_tail")
        for wi in range(8):
            nc.tensor.matmul(fill_ps[:, :], g.idb_sb[:, :], g.wv_sb[0][:, :],
                             start=True, stop=True)
        _emit_stage_b2(g, NSP - 1)


def build_nc():
    nc = bacc.Bacc("TRN2", target_bir_lowering=False, debug=False,
                   enable_asserts=False)
    h = {}
    h["x_t"] = nc.declare_dram_parameter("x_t", [NSP, D, FP], BF16, False)
    h["x8"] = nc.declare_dram_parameter("x8", [NSP, 2, P, 2, FP], F8, False)
    h["m8"] = nc.declare_dram_parameter("m8", [2, P, 2, D], F8, False)
    h["wv"] = nc.declare_dram_parameter("wv", [D, D], BF16, False)
    h["w18"] = nc.declare_dram_parameter("w18", [P, 2, L], F8, False)
    h["w28"] = nc.declare_dram_parameter("w28", [P, 2, 2], F8, False)
    h["p1t"] = nc.declare_dram_parameter("p1t", [CH_PER_CORE, L, FP], BF16, False)
    h["dist"] = nc.declare_dram_parameter("dist", [L, FP], F32, False)
    h["b2r"] = nc.declare_dram_parameter("b2r", [P, 1], F32, False)
    h["identb"] = nc.declare_dram_parameter("identb", [P, P], BF16, False)
    h["out"] = nc.declare_dram_parameter("out", [NSP, FP, D], BF16, True)

    with TileContext(nc) as tc:
        _emit(nc, tc, h)
    nc.compile()
    return nc


def make_in_maps(x, w_qkv, pos_emb, w1, b1, w2, b2, w_out, b_out):
    f8 = mybir.dt.np(F8)
    bf = mybir.dt.np(BF16)
    f = lambda a: np.ascontiguousarray(np.asarray(a), dtype=np.float32)
    x, w_qkv, pos_emb = f(x), f(w_qkv), f(pos_emb)
    w1, b1, w2, b2, w_out, b_out = f(w1), f(b1), f(w2), f(b2), f(w_out), f(b_out)

    wq, wk, wv = w_qkv[:, :D], w_qkv[:, D:2 * D], w_qkv[:, 2 * D:]
    m = (SCALE * (wq.astype(np.float64) @ wk.astype(np.float64).T)
         ).astype(np.float32)
    wvp = (wv.astype(np.float64) @ w_out.astype(np.float64)).astype(bf)
    # fp8 pair-packed 64*M: [dpair, p, ktile, e]
    m8 = np.ascontiguousarray(
        (64.0 * m).reshape(2, 2, P, D).transpose(0, 2, 1, 3).astype(f8))
    # fp8 pair-packed 16*w1: [p, ktile, j]
    w18 = np.ascontiguousarray(
        (16.0 * w1).reshape(2, P, L).transpose(1, 0, 2).astype(f8))
    w28 = np.ascontiguousarray(
        np.repeat((16.0 * w2).reshape(2, P, 1).transpose(1, 0, 2), 2,
                  axis=2).astype(f8))
    # P1[c] = pos[c] @ w1 + b1, transposed [L(j), L(i)] per channel, x256
    p1 = pos_emb[0].astype(np.float64) @ w1.astype(np.float64) + b1
    p1t_single = np.ascontiguousarray((256.0 * p1).transpose(0, 2, 1).astype(bf))
    idx = np.arange(L, dtype=np.float32)
    dist = (idx[None, :] - idx[:, None]) ** 2
    distp = np.ascontiguousarray(np.concatenate([dist, dist], axis=1))
    common = {
        "m8": m8,
        "wv": np.ascontiguousarray(wvp),
        "w18": w18,
        "w28": w28,
        "dist": distp,
        "b2r": np.full((P, 1), b2.reshape(-1)[0], np.float32),
        "identb": np.eye(P, dtype=bf),
    }
    in_maps = []
    for core in range(NCORES):
        x_t = np.empty((NSP, D, FP), np.float32)
        p1t = np.empty((CH_PER_CORE, L, FP), bf)
        for ci in range(CH_PER_CORE):
            ch = core * CH_PER_CORE + ci
            p1t[ci, :, :L] = p1t_single[ch]
            p1t[ci, :, L:] = p1t_single[ch]
            for bp in range(B // 2):
                s = ci * (B // 2) + bp
                x_t[s, :, :L] = x[2 * bp, ch].T
                x_t[s, :, L:] = x[2 * bp + 1, ch].T
        # fp8 pair-packed x^T: [sp, dpair, p, ktile, l]
        x8 = np.ascontiguousarray(
            x_t.reshape(NSP, 2, 2, P, FP).transpose(0, 1, 3, 2, 4).astype(f8))
        mcore = dict(common)
        mcore["x_t"] = np.ascontiguousarray(x_t.astype(bf))
        mcore["x8"] = x8
        mcore["p1t"] = np.ascontiguousarray(p1t)
        in_maps.append(mcore)
    return in_maps


def assemble_out(results, b_out=None):
    """results: list (per core) of dicts with 'out' [NSP, FP(i-packed), D]."""
    y = np.empty((B, C, L, D), np.float32)
    for core in range(NCORES):
        o = np.asarray(results[core]["out"], dtype=np.float32)
        for ci in range(CH_PER_CORE):
            ch = core * CH_PER_CORE + ci
            for bp in range(B // 2):
                s = ci * (B // 2) + bp
                y[2 * bp, ch] = o[s, :L, :]
                y[2 * bp + 1, ch] = o[s, L:, :]
    if b_out is not None:
        y += np.asarray(b_out, np.float32).reshape(1, 1, 1, D)
    return y


_NC = None
LAST_RESULT = None


def kernel(x, w_qkv, pos_emb, w1, b1, w2, b2, w_out, b_out):
    global _NC, LAST_RESULT
    from concourse.bass_utils import run_bass_kernel_spmd

    if _NC is None:
        _NC = build_nc()
    in_maps = make_in_maps(x, w_qkv, pos_emb, w1, b1, w2, b2, w_out, b_out)
    res = run_bass_kernel_spmd(_NC, in_maps, core_ids=list(range(NCORES)))
    LAST_RESULT = res
    return assemble_out(res.results, b_out)
